# revision 2
# baseline (speedup 1.0000x reference)
"""Distributed Trainium2 Bass kernel for nn_AttnHead — v3.

Math (B=2, N=6144, H=256, O=128):
  sf[b,n,:] = seq[b,n,:] @ W.T ; f1 = sf@w1+b1 ; f2 = sf@w2+b2
  logits[b,j,i] = f1[b,i] + f2[b,j]
  coefs = softmax_b(leaky_relu(logits, .01)); c0 = sigma(l0-l1), c1 = 1-c0
  vals[0,i,:] = sum_j c0[j,i] sf[0,j,:] ; vals[1] = S1 - sum_j c0 sf[1]
  out = elu(vals + bias)

Key decomposition (lrelu(x) = x - 0.99 min(x,0)):
  d   = l0 - l1 = D1[i] + D2[j] - 0.99 min(a,0) + 0.99 min(b,0)
  m0x = min(0.99a - D1, -D1) = 0.99 min(a,0) - D1[i]      (DVE stt fused)
  rb  = Relu(-0.99b)        = -0.99 min(b,0)              (ACT Relu / DVE ts)
  -d  = (rb - D2col) + m0x                                 (DVE stt fused)
  c0  = sigma(-1 * (-d))   merged, no bias                 (ACT)
Column scalars (0.99 f2dev, -0.99 f2dev) come straight from the sf
matmul via weight columns baked on the host: wtuu = [W^T | u1 | .99u2 | -.99u2].

Aggregation is o-major (sf tile stationary, c0 moving): vals_T[o,i] per
batch; S1 accumulated by a 1-column ones matmul sharing the b=1
stationary. Epilogue ELU uses S1/bias as per-partition ACT biases.
Output DRAM layout [B, O, N/8]; host transposes.

Sharding: each core gets seqTb rolled so its own 6 i-tiles come first.
Collective-free; sf is computed 8x redundantly (cheap on PE).
"""

import sys

sys.path.insert(0, "/opt/trn_rl_repo")

import numpy as np

from concourse import bacc, mybir, tile
from concourse.bass_utils import run_bass_kernel_spmd
from concourse.masks import make_identity

B, N, H, O, R = 2, 6144, 256, 128, 8
NL = N // R            # 768 local rows
NJT = N // 128         # 48 j-tiles
NW = 131               # W^T cols + u1 + .99u2 + -.99u2
CHW = 6                # j-tiles per chunk
NCH = NJT // CHW       # 8 chunks
SGM = 6                # j-tiles per sigmoid call
F32, BF16 = mybir.dt.float32, mybir.dt.bfloat16
AF = mybir.ActivationFunctionType
ALU = mybir.AluOpType

COPY_MODE = "act"      # 'act'/'dve'/'eng' (split) for psum->sbuf sf copies
RB_ACT_MOD = (0, 2)    # tiles with (jj%3) in this tuple do rb on ACT
RB_POOL_MOD = ()       # tiles with (jj%3) in this tuple do rb on Pool (gpsimd)
NEGY_PE_MOD = ()       # tiles with (jj%3) in this tuple assemble -d+D2 on PE


def build_graph3(reps=1):
    nc = bacc.Bacc("TRN2", target_bir_lowering=False, debug=False, num_devices=R)

    seq_d = nc.dram_tensor("seqTb", [128, 2, B, N], BF16, kind="ExternalInput")
    wt_d = nc.dram_tensor("wtuu", [128, 2, NW], BF16, kind="ExternalInput")
    cst_d = nc.dram_tensor("consts", [4], F32, kind="ExternalInput")
    non_d = nc.dram_tensor("nonce", [1], F32, kind="ExternalInput")
    out_d = nc.dram_tensor("out", [B, O, NL], BF16, kind="ExternalOutput")

    with tile.TileContext(nc) as tc:
      for _rep in range(reps):
        with (
            tc.tile_pool(name="const", bufs=1) as cp,
            tc.tile_pool(name="work", bufs=2) as wk,
            tc.tile_pool(name="mlp", bufs=3) as mlp,
            tc.tile_pool(name="psSF", bufs=2, space="PSUM") as psSF,
            tc.tile_pool(name="psV", bufs=1, space="PSUM") as psV,
            tc.tile_pool(name="psT", bufs=1, space="PSUM") as psT,
        ):
            dmae = [nc.sync, nc.scalar]

            # ---------------- small loads / consts ----------------
            wtuu = cp.tile([128, 2, NW], BF16)
            nc.sync.dma_start(wtuu, wt_d.ap())
            consts = wk.tile([1, 4], F32, tag="consts", bufs=1)
            nc.scalar.dma_start(consts, cst_d.ap().rearrange("(a x) -> a x", a=1))
            noncet = wk.tile([1, 1], F32, tag="noncet", bufs=1)
            nc.scalar.dma_start(noncet, non_d.ap().rearrange("(a x) -> a x", a=1))
            id16 = cp.tile([128, 128], BF16)
            make_identity(nc, id16)
            onesrow = cp.tile([1, 128], BF16)
            nc.vector.memset(onesrow, 1.0)
            ones_col = cp.tile([128, 1], BF16)
            nc.vector.memset(ones_col, 1.0)

            # cbb = b1 + b2
            cbb = wk.tile([1, 1], F32, tag="cbb", bufs=1)
            nc.vector.tensor_tensor(cbb, consts[:, 0:1], consts[:, 1:2], ALU.add)

            # ---------------- persistent SBUF ----------------
            sfg = cp.tile([128, B, NJT, NW], BF16)
            q0f = cp.tile([128, NJT], F32)    # .99 * f2dev[0]
            qb1f = cp.tile([128, NJT], F32)   # -.99 * f2dev[1]
            d2g = cp.tile([128, NJT], F32)    # D2 = f2dev0 - f2dev1
            P0xb = cp.tile([128, NL], BF16)
            ND1b = cp.tile([128, NL], BF16)
            NP1zb = cp.tile([128, NL], BF16)

            # vals psum: A/B = batch0 (i 0:512 / 512:768), C/D = batch1 (+ s1)
            vA = psV.tile([128, 512], F32, name="vA")
            vB = psV.tile([128, 256], F32, name="vB")
            vC = psV.tile([128, 512], F32, name="vC")
            vD = psV.tile([128, 256], F32, name="vD")
            s1p = psV.tile([128, 1], F32, name="s1p")

            st_tiles = {}

            # ---------------- per-chunk sf stage ----------------
            def sf_stage(c):
                n0 = c * CHW * 128
                st = mlp.tile([128, 2, B, CHW * 128], BF16, tag="st", bufs=2)
                st_tiles[c] = st
                dmae[c % 2].dma_start(st, seq_d.ap()[:, :, :, n0 : n0 + CHW * 128])
                for b in range(B):
                    for tr in range(2):  # triples of j-tiles
                        sp = psSF.tile([128, 3 * NW], F32, tag="sf")
                        for k in range(3):
                            nb = tr * 3 + k
                            for hc in range(2):
                                nc.tensor.matmul(
                                    sp[:, k * NW : (k + 1) * NW],
                                    st[:, hc, b, nb * 128 : (nb + 1) * 128],
                                    wtuu[:, hc],
                                    start=(hc == 0),
                                    stop=(hc == 1),
                                )
                        jj0 = c * CHW + tr * 3
                        dst = sfg[:, b, jj0 : jj0 + 3].rearrange("p t w -> p (t w)")
                        if COPY_MODE == "act" or (
                            COPY_MODE == "eng" and (b + tr) % 2 == 0
                        ):
                            nc.scalar.activation(dst, sp, AF.Identity)
                        else:
                            nc.vector.tensor_copy(dst, sp)
                # q columns for this chunk (f32) + D2
                cs = c * CHW
                nc.scalar.activation(
                    q0f[:, cs : cs + CHW], sfg[:, 0, cs : cs + CHW, 129], AF.Identity
                )
                nc.scalar.activation(
                    qb1f[:, cs : cs + CHW], sfg[:, 1, cs : cs + CHW, 130], AF.Identity
                )
                t2 = wk.tile([128, CHW], F32, tag="t2")
                nc.vector.tensor_tensor(
                    t2, q0f[:, cs : cs + CHW], qb1f[:, cs : cs + CHW], ALU.add
                )
                nc.vector.tensor_scalar_mul(d2g[:, cs : cs + CHW], t2, 1.0 / 0.99)

            sf_stage(0)

            # ------------- rows prep: f1dev rows via u1-col matmuls -------------
            # frows[0, b, n] = f1dev[b, local n] = sum_h u1[h] seqT[h, n]
            frows = wk.tile([1, B, NL], F32, tag="frows", bufs=1)
            pbc = psT.tile([128, 512], F32, name="pbc")
            pfr = pbc[0:1, :]
            st0 = st_tiles[0]
            for b in range(B):
                for o0, w in ((0, 512), (512, 256)):
                    for hc in range(2):
                        nc.tensor.matmul(
                            pfr[:, :w],
                            wtuu[:, hc, 128:129],
                            st0[:, hc, b, o0 : o0 + w],
                            start=(hc == 0),
                            stop=(hc == 1),
                        )
                    nc.vector.tensor_copy(frows[:, b, o0 : o0 + w], pfr[:, :w])

            d1r = wk.tile([1, NL], BF16, tag="d1r", bufs=1)
            nc.vector.tensor_tensor(d1r, frows[:, 0], frows[:, 1], ALU.subtract)
            nd1r = wk.tile([1, NL], BF16, tag="nd1r", bufs=1)
            nc.vector.tensor_scalar_mul(nd1r, d1r, -1.0)
            p0raw = wk.tile([1, NL], BF16, tag="p0raw", bufs=1)
            nc.vector.scalar_tensor_tensor(
                p0raw, frows[:, 0], 0.99, d1r, ALU.mult, ALU.subtract
            )
            cbb99 = wk.tile([1, 1], F32, tag="cbb99", bufs=1)
            nc.vector.tensor_scalar_mul(cbb99, cbb, 0.99)
            p0xr = wk.tile([1, NL], BF16, tag="p0xr", bufs=1)
            nc.vector.tensor_scalar(p0xr, p0raw, cbb99[:, 0:1], None, ALU.add)
            np1zr = wk.tile([1, NL], BF16, tag="np1zr", bufs=1)
            nc.vector.tensor_scalar(
                np1zr, frows[:, 1], cbb[:, 0:1], -0.99, ALU.add, ALU.mult
            )

            # broadcasts via PE ones-outer-product
            for row, dstb in ((p0xr, P0xb), (nd1r, ND1b), (np1zr, NP1zb)):
                nc.tensor.matmul(pbc, onesrow, row[:, :512], start=True, stop=True)
                nc.scalar.activation(dstb[:, :512], pbc, AF.Identity)
                nc.tensor.matmul(
                    pbc[:, :256], onesrow, row[:, 512:NL], start=True, stop=True
                )
                nc.scalar.activation(dstb[:, 512:NL], pbc[:, :256], AF.Identity)

            # ---------------- main loop ----------------
            for c in range(NCH):
                if c + 1 < NCH:
                    sf_stage(c + 1)
                for g in range(CHW // SGM):
                    dd = mlp.tile([128, SGM * NL], BF16, tag="dd", bufs=3)
                    c0 = mlp.tile([128, SGM * NL], BF16, tag="c0", bufs=3)
                    for t in range(SGM):
                        jj = c * CHW + g * SGM + t
                        m0 = mlp.tile([128, NL], BF16, tag="m0")
                        nc.vector.scalar_tensor_tensor(
                            m0, P0xb, q0f[:, jj : jj + 1], ND1b, ALU.add, ALU.min
                        )
                        if jj % 3 in RB_POOL_MOD:
                            rb = mlp.tile([128, NL], BF16, tag="rbp", bufs=6)
                        else:
                            rb = mlp.tile([128, NL], BF16, tag="rb", bufs=4)
                        if jj % 3 in RB_ACT_MOD:
                            nc.scalar.activation(
                                rb, NP1zb, AF.Relu, bias=qb1f[:, jj : jj + 1]
                            )
                        elif jj % 3 in RB_POOL_MOD:
                            nc.gpsimd.tensor_scalar(
                                rb, NP1zb, qb1f[:, jj : jj + 1], 0.0, ALU.add,
                                ALU.max,
                            )
                        else:
                            nc.vector.tensor_scalar(
                                rb, NP1zb, qb1f[:, jj : jj + 1], 0.0, ALU.add,
                                ALU.max,
                            )
                        if jj % 3 in NEGY_PE_MOD:
                            # PE assembles rb+m0x (= -d + D2); sigma applies
                            # scale=-1 and bias=+D2 per 512/256 split
                            for yp, c0l, c0h in ((yp5, 0, 512), (yp2, 512, NL)):
                                w = c0h - c0l
                                nc.tensor.matmul(
                                    yp[:, :w], id16, rb[:, c0l:c0h],
                                    start=True, stop=False,
                                )
                                nc.tensor.matmul(
                                    yp[:, :w], id16, m0[:, c0l:c0h],
                                    start=False, stop=True,
                                )
                                nc.scalar.activation(
                                    c0[:, t * NL + c0l : t * NL + c0h],
                                    yp[:, :w],
                                    AF.Sigmoid,
                                    scale=-1.0,
                                    bias=d2g[:, jj : jj + 1],
                                )
                        else:
                            nc.vector.scalar_tensor_tensor(
                                dd[:, t * NL : (t + 1) * NL],
                                rb,
                                d2g[:, jj : jj + 1],
                                m0,
                                ALU.subtract,
                                ALU.add,
                            )
                    ddtiles = [
                        t for t in range(SGM)
                        if (c * CHW + g * SGM + t) % 3 not in NEGY_PE_MOD
                    ]
                    if ddtiles and ddtiles == list(
                        range(ddtiles[0], ddtiles[0] + len(ddtiles))
                    ):
                        lo, hi = ddtiles[0] * NL, (ddtiles[-1] + 1) * NL
                        nc.scalar.activation(
                            c0[:, lo:hi], dd[:, lo:hi], AF.Sigmoid, scale=-1.0
                        )
                    else:
                        for t in ddtiles:
                            nc.scalar.activation(
                                c0[:, t * NL : (t + 1) * NL],
                                dd[:, t * NL : (t + 1) * NL],
                                AF.Sigmoid,
                                scale=-1.0,
                            )
                    for t in range(SGM):
                        jj = c * CHW + g * SGM + t
                        cs = t * NL
                        first, last = (jj == 0), (jj == NJT - 1)
                        nc.tensor.matmul(
                            vA, sfg[:, 0, jj, :128], c0[:, cs : cs + 512],
                            start=first, stop=last,
                        )
                        nc.tensor.matmul(
                            vB, sfg[:, 0, jj, :128], c0[:, cs + 512 : cs + NL],
                            start=first, stop=last,
                        )
                        nc.tensor.matmul(
                            vC, sfg[:, 1, jj, :128], c0[:, cs : cs + 512],
                            start=first, stop=last,
                        )
                        nc.tensor.matmul(
                            vD, sfg[:, 1, jj, :128],
                            c0[:, cs + 512 : cs + NL],
                            start=first, stop=last,
                        )
                        nc.tensor.matmul(
                            s1p, sfg[:, 1, jj, :128], ones_col,
                            start=first, stop=last,
                        )

            # ---------------- epilogue ----------------
            s1c = wk.tile([128, 1], F32, tag="s1c", bufs=1)
            nc.vector.tensor_copy(s1c, s1p)
            biascol = wk.tile([128, 1], F32, tag="biascol", bufs=1)
            nc.gpsimd.partition_broadcast(biascol, consts[:, 2:3])
            sb1 = wk.tile([128, 1], F32, tag="sb1", bufs=1)   # S1 + bias
            nc.vector.tensor_tensor(sb1, s1c, biascol, ALU.add)
            nsb1 = wk.tile([128, 1], F32, tag="nsb1", bufs=1)
            nc.vector.tensor_scalar_mul(nsb1, sb1, -1.0)
            nbias = wk.tile([128, 1], F32, tag="nbias", bufs=1)
            nc.vector.tensor_scalar_mul(nbias, biascol, -1.0)

            # elu(x) = relu(x) + exp(min(x,0)) - 1
            #   b0: x = v + bias ; b1: x = S1 + bias - v
            for b in range(B):
                rp = mlp.tile([128, NL], BF16, tag="rp")
                nm = mlp.tile([128, NL], BF16, tag="nm")
                ev = mlp.tile([128, NL], BF16, tag="ev")
                ot = mlp.tile([128, NL], BF16, tag="ot")
                pieces = (
                    ((vA, 0, 512), (vB, 512, 256))
                    if b == 0
                    else ((vC, 0, 512), (vD, 512, 256))
                )
                for src, o0, w in pieces:
                    sl = slice(o0, o0 + w)
                    if b == 0:
                        nc.scalar.activation(
                            rp[:, sl], src[:, :w], AF.Relu, bias=biascol
                        )
                        nc.scalar.activation(
                            nm[:, sl], src[:, :w], AF.Relu, bias=nbias, scale=-1.0
                        )
                    else:
                        nc.scalar.activation(
                            rp[:, sl], src[:, :w], AF.Relu, bias=sb1, scale=-1.0
                        )
                        nc.scalar.activation(
                            nm[:, sl], src[:, :w], AF.Relu, bias=nsb1
                        )
                nc.scalar.activation(ev, nm, AF.Exp, scale=-1.0)
                nc.vector.scalar_tensor_tensor(
                    ot, ev, -1.0, rp, ALU.add, ALU.add
                )
                dmae[b].dma_start(out_d.ap()[b], ot)

    nc.compile()
    return nc


def make_in_maps3(inputs):
    seq = np.asarray(inputs["seq"], dtype=np.float32)          # [B, N, H]
    W = np.asarray(inputs["W_fts"], dtype=np.float32)          # [O, H]
    w1 = np.asarray(inputs["w1"], dtype=np.float32)
    w2 = np.asarray(inputs["w2"], dtype=np.float32)
    b1 = float(np.asarray(inputs["b1"]).reshape(-1)[0])
    b2 = float(np.asarray(inputs["b2"]).reshape(-1)[0])
    bias = float(np.asarray(inputs["bias"]).reshape(-1)[0])

    import ml_dtypes

    bf = ml_dtypes.bfloat16
    # seqT [h, b, n] -> [p, hc, b, n]
    seqT = seq.transpose(2, 0, 1)                              # [H, B, N]
    seqTb = np.ascontiguousarray(
        seqT.reshape(2, 128, B, N).transpose(1, 0, 2, 3).astype(bf)
    )                                                          # [128, hc, B, N]
    u1 = w1 @ W                                                # [H]
    u2 = w2 @ W
    wt = np.zeros((128, 2, NW), dtype=np.float64)
    WT = W.T.reshape(2, 128, O).transpose(1, 0, 2)             # [p, hc, O]
    wt[:, :, :O] = WT
    wt[:, :, O] = u1.reshape(2, 128).T
    wt[:, :, O + 1] = (0.99 * u2).reshape(2, 128).T
    wt[:, :, O + 2] = (-0.99 * u2).reshape(2, 128).T
    wtuu = np.ascontiguousarray(wt.astype(bf))
    consts = np.array([b1, b2, bias, 0.0], dtype=np.float32)

    in_maps = []
    for r in range(R):
        m = {
            "seqTb": np.ascontiguousarray(np.roll(seqTb, -r * NL, axis=3)),
            "wtuu": wtuu,
            "consts": consts,
            "nonce": np.zeros(1, dtype=np.float32),
        }
        in_maps.append(m)
    return in_maps


def gather_out3(res) -> np.ndarray:
    shards = [
        np.asarray(res.results[r]["out"]).astype(np.float32) for r in range(R)
    ]
    full = np.concatenate(shards, axis=2)                      # [B, O, N]
    return np.ascontiguousarray(full.transpose(0, 2, 1))       # [B, N, O]


_NC_CACHE = None


def kernel(**inputs) -> np.ndarray:
    global _NC_CACHE
    if _NC_CACHE is None:
        _NC_CACHE = build_graph3()
    res = run_bass_kernel_spmd(
        _NC_CACHE, make_in_maps3(inputs), core_ids=list(range(R))
    )
    return gather_out3(res)
